# revision 43
# baseline (speedup 1.0000x reference)
"""Trainium2 Bass kernel for factorized space-time attention.

Computation (per batch b of 8, one NeuronCore each):
  qkv = x @ w_qkv.T                      (3136, 2304)
  heads 0-5:  spatial attention over 196 patches within each of 16 frames
  heads 6-11: temporal attention over groups of 16 consecutive tokens
              (raw-reshape semantics of the reference)
  out = concat(head outputs) @ w_proj.T + b_proj

Strategy: data-parallel over batch (8 cores). All activations kept
feature-major ([d, n]) on chip so every matmul contraction runs over the
partition dim with no on-device transposes; x / weights are pre-transposed
host-side and cast to bf16 (PE runs bf16 at 4x the fp32 rate; tolerance is
2e-2 so bf16 rounding is comfortably inside budget).

Attention works on head PAIRS so normalization is partition-aligned.
Matmul PSUM outputs must start at a 1024-byte (256 fp32 column) boundary
inside their bank (an unaligned column offset crashes the device), so
pair tiles pack two regions per bank at column offsets 0 and 256:
  ps_st  [msz, 512]  scores: even head @0, odd head @256 (one exp inst
                     reads both via a strided AP)
  ps_avd [128, 512]  AV numerators @0 (rows 0:64 even / 64:128 odd) and
                     softmax denominators @256, broadcast to the same rows
                     via an extra ones-matmul
  one DVE reciprocal [128, L] + one DVE multiply write the normalized pair
  straight into attnT — no broadcast matmul, no scalar-engine staging copy,
  no partition-shift DMA.
The temporal block-diagonal mask is folded into the score matmuls as a
rank-8 accumulated matmul (score += -320 outside the diagonal blocks), so
exp() gives ~0 there and no separate mask multiply is needed.
Softmax skips the max-subtraction (scores are ~N(0,1); exp is safe).
"""

import sys

# concourse normally comes from the axon site tree (sitecustomize); the
# append is a fallback so a bare environment still finds it.
if "/opt/trn_rl_repo" not in sys.path:
    sys.path.append("/opt/trn_rl_repo")

import numpy as np

import concourse.bass as bass  # noqa: F401  (engine namespaces live on nc)
from concourse.bass import _add_dep_helper
import concourse.mybir as mybir
import concourse.tile as tile
from concourse import bacc
from concourse.bass_utils import run_bass_kernel_spmd

F32 = mybir.dt.float32
BF16 = mybir.dt.bfloat16
AF = mybir.ActivationFunctionType

# problem dims (hardcoded per contract)
B = 8
F = 16
P = 196
D = 768
NH = 12
HD = 64
N = F * P  # 3136
E3 = 3 * D  # 2304
SB = 784  # superblock = lcm(196, 16) tokens
NSB = N // SB  # 4
FPSB = SB // P  # 4 frames per superblock
WPSB = SB // 112  # 7 temporal windows per superblock
SCALE = HD ** -0.5
MASKVAL = -320.0  # pre-scale additive mask; exp(SCALE*-320) == 0 in bf16

# compute dtype for matmul inputs ("f32" safest, "bf16" 4x faster on PE)
COMPUTE = "bf16"

_CACHE = {}


def _build(compute: str, reps: int = 1, ncores: int = B):
    """Build + bass-compile the per-core kernel. Returns the Bacc object.

    compute: "f32" | "bf16" — dtype of all matmul inputs.
    reps: device-side repetition count (for timing; wraps the body in For_i).
    """
    cdt = BF16 if compute == "bf16" else F32

    nc = bacc.Bacc("TRN2", target_bir_lowering=False, debug=False,
                   num_devices=ncores)

    xt_d = nc.dram_tensor("xt", (D, N), cdt, kind="ExternalInput")
    wqkv_d = nc.dram_tensor("wqkvT", (D, E3), cdt, kind="ExternalInput")
    wproj_d = nc.dram_tensor("wprojT", (D, D), cdt, kind="ExternalInput")
    bias_d = nc.dram_tensor("bias", (D, 1), F32, kind="ExternalInput")
    # rank-8 factorization of the temporal block-diag mask:
    # maskM = mml.T @ mmr  ([112,112], 0 on diag blocks, -320 off)
    mml_d = nc.dram_tensor("mml", (8, 112), cdt, kind="ExternalInput")
    mmr_d = nc.dram_tensor("mmr", (8, 112), cdt, kind="ExternalInput")
    out_d = nc.dram_tensor("outT", (D, N), F32, kind="ExternalOutput")

    with tile.TileContext(nc) as tc:
        with (
            tc.tile_pool(name="const", bufs=1) as cpool,
            tc.tile_pool(name="work", bufs=1) as wpool,
            tc.tile_pool(name="small", bufs=4) as spool,
            tc.tile_pool(name="psum", bufs=2, space="PSUM") as ppool,
        ):
            # ---- constants -------------------------------------------------
            wq = []
            for dc in range(6):
                t = cpool.tile([128, E3], cdt, tag=f"wq{dc}", name=f"wq{dc}")
                nc.sync.dma_start(t[:], wqkv_d.ap()[128 * dc : 128 * (dc + 1), :])
                wq.append(t)
            wp = []
            for dc in range(6):
                t = cpool.tile([128, D], cdt, tag=f"wp{dc}", name=f"wp{dc}")
                nc.sync.dma_start(t[:], wproj_d.ap()[128 * dc : 128 * (dc + 1), :])
                wp.append(t)
            bias_t = cpool.tile([128, 6], F32, tag="bias", name="bias_t")
            nc.sync.dma_start(
                bias_t[:], bias_d.ap().rearrange("(e p) one -> p (e one)", p=128)
            )
            mml_t = cpool.tile([8, 112], cdt, tag="mml", name="mml_t")
            nc.sync.dma_start(mml_t[:], mml_d.ap())
            mmr_t = cpool.tile([8, 112], cdt, tag="mmr", name="mmr_t")
            nc.sync.dma_start(mmr_t[:], mmr_d.ap())
            zeros_col = cpool.tile([128, 1], F32, tag="zeros_c", name="zeros_col")
            nc.gpsimd.memset(zeros_col[:], 0.0)
            # all-ones stationary operand of the denominator matmuls
            ones_t = cpool.tile([128, 64], cdt, tag="ones", name="ones_t")
            nc.gpsimd.memset(ones_t[:], 1.0)

            import contextlib

            rep_ctx = tc.For_i(0, reps, 1) if reps > 1 else contextlib.nullcontext()
            with rep_ctx:
              for s in range(NSB):
                so = SB * s  # superblock token offset

                # ---- load x^T superblock ----------------------------------
                xts = []
                for dc in range(6):
                    t = wpool.tile([128, SB], cdt, tag=f"xts{dc}", bufs=2,
                                   name=f"xts{dc}_{s}")
                    nc.sync.dma_start(
                        t[:], xt_d.ap()[128 * dc : 128 * (dc + 1), so : so + SB]
                    )
                    xts.append(t)

                # ---- QKV projection: Q,K regions, feature-major -----------
                # qkvt[t] rows = features 128t..128t+127 of [Q(768) | K(768)]
                qkvt = []
                for ti in range(12):
                    qt = wpool.tile([128, SB], cdt, tag=f"qkvt{ti}", bufs=2,
                                    name=f"qkvt{ti}_{s}")
                    for j in range(2):
                        ps = ppool.tile([128, 392], F32, tag="mm", bufs=2,
                                        name=f"ps_qk{s}_{ti}_{j}")
                        for dc in range(6):
                            nc.tensor.matmul(
                                ps[:],
                                wq[dc][:, 128 * ti : 128 * (ti + 1)],
                                xts[dc][:, 392 * j : 392 * (j + 1)],
                                start=(dc == 0),
                                stop=(dc == 5),
                            )
                        nc.vector.tensor_copy(qt[:, 392 * j : 392 * (j + 1)], ps[:])
                    qkvt.append(qt)

                # ---- V projection, token-major (natural) ------------------
                # per tile: 6 heads x 64 V-cols = 384 cols
                def v_proj(msz, tok0, wcol0, vtag, vname, psname):
                    vt_ = wpool.tile([msz, 384], cdt, tag=vtag, bufs=2, name=vname)
                    ps = ppool.tile([msz, 384], F32, tag="mm", bufs=2, name=psname)
                    for dc in range(6):
                        nc.tensor.matmul(
                            ps[:],
                            xts[dc][:, tok0 : tok0 + msz],
                            wq[dc][:, wcol0 : wcol0 + 384],
                            start=(dc == 0),
                            stop=(dc == 5),
                        )
                    nc.scalar.copy(vt_[:], ps[:])
                    return vt_

                # spatial V: per-frame chunks of [128+68] rows; cols = heads 0-5
                vs = []
                for f in range(FPSB):
                    for ci, (m0, msz) in enumerate(((0, 128), (128, 68))):
                        vs.append(
                            v_proj(msz, 196 * f + m0, 1536, f"vs{f}_{ci}",
                                   f"vs{f}_{ci}_{s}", f"ps_vs{s}_{f}_{ci}")
                        )
                # temporal V: uniform 112-token windows; cols = heads 6-11
                vt = []
                for w in range(WPSB):
                    vt.append(
                        v_proj(112, 112 * w, 1920, f"vt{w}",
                               f"vt{w}_{s}", f"ps_vt{s}_{w}")
                    )

                # ---- attention output, feature-major ----------------------
                attnT = [
                    wpool.tile([128, SB], cdt, tag=f"attnT{i}", bufs=2,
                               name=f"attnT{i}_{s}")
                    for i in range(6)
                ]

                def pair_norm(ps_avd, at, cols, L, name):
                    """reciprocal of den + one multiply -> attnT columns.

                    Data deps keep this bank-collision safe: the matmul
                    chain ends with the den groups, reciprocal RAW-depends
                    on them, and the multiply RAW-depends on rb — so no
                    engine reads the bank while the PE still writes it.
                    """
                    rb = spool.tile([128, L], F32, tag="rb", name=f"rb{name}")
                    nc.vector.reciprocal(rb[:], ps_avd[:, 256 : 256 + L])
                    nc.vector.tensor_mul(at[:, cols], ps_avd[:, 0:L], rb[:])

                # chain same-bank psum groups sequentially: a group's
                # start=True clears the bank's has_written bits, so it must
                # not begin until the previous group in that bank stopped.
                def chain(mm, prev):
                    if prev is not None:
                        _add_dep_helper(mm.ins, prev.ins, sync=False,
                                        reason="sequential psum groups in bank")
                    return mm

                # ---- spatial attention (heads 0-5, by pair, per frame) -----
                import os

                _abl = os.environ.get("KABL", "st")
                _slvl = 3 if "s" in _abl else 0
                for p in _abl.split(","):
                    if p.startswith("s") and p[1:].isdigit():
                        _slvl = int(p[1:])
                if _abl != "st":
                    for t in attnT:
                        nc.gpsimd.memset(t[:], 0.0)
                for f in range(FPSB) if _slvl >= 1 else []:
                    fo = 196 * f
                    for hp in range(3):
                        qtile = qkvt[hp]       # Q features, heads 2hp/2hp+1
                        ktile = qkvt[6 + hp]   # K features
                        es = {}
                        for ci, (m0, msz) in enumerate(((0, 128), (128, 68))):
                            ps_st = ppool.tile(
                                [msz, 512], F32, tag="st", bufs=3,
                                name=f"ps_st{s}_{f}_{hp}_{ci}",
                            )
                            # HW erratum: a K<128 matmul with >64 weight
                            # columns crashes when its PSUM output starts at
                            # a column offset, so matmuls at offset 256 are
                            # split into <=64-weight sub-matmuls.
                            for hi in range(2):
                                pb = 64 * hi
                                co = 256 * hi
                                pieces = (
                                    ((0, msz),) if hi == 0
                                    else ((0, 64), (64, msz - 64))
                                )
                                for c0, csz in pieces:
                                    nc.tensor.matmul(
                                        ps_st[c0 : c0 + csz, co : co + 196],
                                        ktile[pb : pb + 64,
                                              fo + m0 + c0 : fo + m0 + c0 + csz],
                                        qtile[pb : pb + 64, fo : fo + 196],
                                        start=True,
                                        stop=True,
                                        skip_group_check=(c0 > 0),
                                    )
                            e = spool.tile(
                                [msz, 392], cdt, tag="e", bufs=8,
                                name=f"e{s}_{f}_{hp}_{ci}",
                            )
                            nc.scalar.activation(
                                e.rearrange("p (g c) -> p g c", c=196),
                                ps_st.rearrange("p (g c) -> p g c", c=256)[
                                    :, :, 0:196
                                ],
                                AF.Exp,
                                bias=zeros_col[:msz, :], scale=SCALE,
                            )
                            es[ci] = e
                        if _slvl < 2:
                            continue
                        ps_avd = ppool.tile([128, 512], F32, tag="avd", bufs=3,
                                            name=f"ps_sav{s}_{f}_{hp}")
                        prev = None  # last matmul of the previous group
                        for kind in range(2):  # 0=num @0, 1=den @256
                            for hi in range(2):
                                h = 2 * hp + hi
                                rs = slice(64 * hi, 64 * hi + 64)
                                co = 256 * kind
                                for ci, (m0, msz) in enumerate(((0, 128), (128, 68))):
                                    lhsT = (
                                        vs[2 * f + ci][:, 64 * h : 64 * h + 64]
                                        if kind == 0
                                        else ones_t[:msz, :]
                                    )
                                    mm = nc.tensor.matmul(
                                        ps_avd[rs, co : co + 196],
                                        lhsT,
                                        es[ci][:msz, 196 * hi : 196 * hi + 196],
                                        start=(ci == 0),
                                        stop=(ci == 1),
                                        skip_group_check=(hi == 1),
                                    )
                                    if ci == 0:
                                        chain(mm, prev)
                                    if ci == 1:
                                        prev = mm
                        if _slvl < 3:
                            continue
                        pair_norm(ps_avd, attnT[hp],
                                  slice(fo, fo + 196), 196, f"s{s}_{f}_{hp}")

                # ---- temporal attention (heads 6-11, per 112-window) -------
                # block-diag mask folded into the score matmul (rank-8 term)
                for w in range(WPSB) if "t" in _abl else []:
                    wo = 112 * w
                    for hp in range(3):
                        ps_st = ppool.tile(
                            [112, 512], F32, tag="st", bufs=3,
                            name=f"ps_tst{s}_{w}_{hp}",
                        )
                        prev = None
                        for hi in range(2):
                            pb = 64 * hi
                            co = 256 * hi
                            # offset-256 matmuls split to <=64 weight cols
                            # (see spatial erratum comment)
                            pieces = (
                                ((0, 112),) if hi == 0 else ((0, 64), (64, 48))
                            )
                            for c0, csz in pieces:
                                mm = nc.tensor.matmul(
                                    ps_st[c0 : c0 + csz, co : co + 112],
                                    qkvt[9 + hp][pb : pb + 64,
                                                 wo + c0 : wo + c0 + csz],
                                    qkvt[3 + hp][pb : pb + 64, wo : wo + 112],
                                    start=True,
                                    stop=False,
                                    skip_group_check=(c0 > 0),
                                )
                                chain(mm, prev)
                                prev = nc.tensor.matmul(
                                    ps_st[c0 : c0 + csz, co : co + 112],
                                    mml_t[:, c0 : c0 + csz],
                                    mmr_t[:],
                                    start=False,
                                    stop=True,
                                    skip_group_check=(c0 > 0),
                                )
                        em = spool.tile(
                            [112, 224], cdt, tag="e", bufs=8,
                            name=f"et{s}_{w}_{hp}",
                        )
                        nc.scalar.activation(
                            em.rearrange("p (g c) -> p g c", c=112),
                            ps_st.rearrange("p (g c) -> p g c", c=256)[:, :, 0:112],
                            AF.Exp,
                            bias=zeros_col[:112, :], scale=SCALE,
                        )
                        ps_avd = ppool.tile([128, 512], F32, tag="avd", bufs=3,
                                            name=f"ps_tav{s}_{w}_{hp}")
                        prev = None
                        for kind in range(2):  # 0=num @0, 1=den @256
                            for hi in range(2):
                                h = 6 + 2 * hp + hi
                                rs = slice(64 * hi, 64 * hi + 64)
                                co = 256 * kind
                                lhsT = (
                                    vt[w][:, 64 * (h - 6) : 64 * (h - 6) + 64]
                                    if kind == 0
                                    else ones_t[:112, :]
                                )
                                mm = nc.tensor.matmul(
                                    ps_avd[rs, co : co + 112],
                                    lhsT,
                                    em[:112, 112 * hi : 112 * hi + 112],
                                    start=True,
                                    stop=True,
                                    skip_group_check=(hi == 1),
                                )
                                chain(mm, prev)
                                prev = mm
                        pair_norm(ps_avd, attnT[3 + hp],
                                  slice(wo, wo + 112), 112, f"t{s}_{w}_{hp}")

                # ---- output projection + bias ------------------------------
                for ec in range(6):
                    for j in range(2):
                        ps = ppool.tile([128, 392], F32, tag="mm", bufs=2,
                                        name=f"ps_o{s}_{ec}_{j}")
                        for dc in range(6):
                            nc.tensor.matmul(
                                ps[:],
                                wp[dc][:, 128 * ec : 128 * (ec + 1)],
                                attnT[dc][:, 392 * j : 392 * (j + 1)],
                                start=(dc == 0),
                                stop=(dc == 5),
                            )
                        ot = spool.tile([128, 392], F32, tag="ot",
                                        name=f"ot{s}_{ec}_{j}")
                        nc.scalar.activation(
                            ot[:], ps[:], AF.Identity,
                            bias=bias_t[:, ec : ec + 1], scale=1.0,
                        )
                        nc.sync.dma_start(
                            out_d.ap()[
                                128 * ec : 128 * (ec + 1),
                                so + 392 * j : so + 392 * (j + 1),
                            ],
                            ot[:],
                        )

    nc.compile()
    return nc


def _get_nc(compute: str):
    if compute not in _CACHE:
        _CACHE[compute] = _build(compute)
    return _CACHE[compute]


def _np_dtype(compute: str):
    if compute == "f32":
        return np.float32
    import ml_dtypes

    return ml_dtypes.bfloat16


def _mask_factors(dt):
    """Rank-8 factorization of the additive temporal mask.

    maskM[k,q] = MASKVAL * (1 - same_group(k,q))
               = MASKVAL*1*1 + sum_g (-MASKVAL)*ind[g,k]*ind[g,q]
    """
    ind = np.zeros((7, 112), np.float32)
    for g in range(7):
        ind[g, 16 * g : 16 * (g + 1)] = 1.0
    mml = np.concatenate([np.full((1, 112), MASKVAL, np.float32), -MASKVAL * ind])
    mmr = np.concatenate([np.ones((1, 112), np.float32), ind])
    return mml.astype(dt), mmr.astype(dt)


def _host_inputs(x, w_qkv, w_proj, b_proj, dt):
    """Full inputs -> per-core in_maps (data-parallel over batch)."""
    x = np.asarray(x, dtype=np.float32).reshape(B, N, D)
    xT = np.ascontiguousarray(x.transpose(0, 2, 1)).astype(dt)  # (B, D, N)
    wqkvT = np.ascontiguousarray(np.asarray(w_qkv, np.float32).T).astype(dt)
    wprojT = np.ascontiguousarray(np.asarray(w_proj, np.float32).T).astype(dt)
    bias = np.asarray(b_proj, np.float32).reshape(D, 1)
    mml, mmr = _mask_factors(dt)
    return [
        {"xt": xT[b], "wqkvT": wqkvT, "wprojT": wprojT, "bias": bias,
         "mml": mml, "mmr": mmr}
        for b in range(B)
    ]


def kernel(x, w_qkv, w_proj, b_proj):
    nc = _get_nc(COMPUTE)
    dt = _np_dtype(COMPUTE)
    in_maps = _host_inputs(x, w_qkv, w_proj, b_proj, dt)
    res = run_bass_kernel_spmd(nc, in_maps, core_ids=list(range(B)))
    out = np.stack([r["outT"].T for r in res.results])  # (B, N, D)
    return np.ascontiguousarray(out.reshape(B, F, P, D)).astype(np.float32)


if __name__ == "__main__":
    rng = np.random.default_rng(0)
    x = rng.standard_normal((B, F, P, D), dtype=np.float32)
    w_qkv = rng.standard_normal((E3, D), dtype=np.float32) * D**-0.5
    w_proj = rng.standard_normal((D, D), dtype=np.float32) * D**-0.5
    b_proj = np.zeros(D, np.float32)
    out = kernel(x=x, w_qkv=w_qkv, w_proj=w_proj, b_proj=b_proj)
    print(out.shape, out.dtype)


# revision 50
# speedup vs baseline: 1.2524x; 1.2524x over previous
"""Trainium2 Bass kernel for factorized space-time attention.

Computation (per batch b of 8, one NeuronCore each):
  qkv = x @ w_qkv.T                      (3136, 2304)
  heads 0-5:  spatial attention over 196 patches within each of 16 frames
  heads 6-11: temporal attention over groups of 16 consecutive tokens
              (raw-reshape semantics of the reference)
  out = concat(head outputs) @ w_proj.T + b_proj

Strategy: data-parallel over batch (8 cores). All activations kept
feature-major ([d, n]) on chip so every matmul contraction runs over the
partition dim with no on-device transposes; x / weights are pre-transposed
host-side and cast to bf16 (PE runs bf16 at 4x the fp32 rate; tolerance is
2e-2 so bf16 rounding is comfortably inside budget).

Attention works on head PAIRS so normalization is partition-aligned.
Matmul PSUM outputs must start at a 1024-byte (256 fp32 column) boundary
inside their bank (an unaligned column offset crashes the device), so
pair tiles pack two regions per bank at column offsets 0 and 256:
  ps_st  [msz, 512]  scores: even head @0, odd head @256 (one exp inst
                     reads both via a strided AP)
  ps_avd [128, 512]  AV numerators @0 (rows 0:64 even / 64:128 odd) and
                     softmax denominators @256, broadcast to the same rows
                     via an extra ones-matmul
  one DVE reciprocal [128, L] + one DVE multiply write the normalized pair
  straight into attnT — no broadcast matmul, no scalar-engine staging copy,
  no partition-shift DMA.
Temporal attention exp()s the raw 112x112 window scores, then one DVE
multiply with the replicated block-diagonal 0/1 mask zeroes the
off-diagonal groups before the AV / denominator matmuls.
Softmax skips the max-subtraction (scores are ~N(0,1); exp is safe).
HW erratum found while packing: a K<128 matmul with >64 weight columns
crashes the device when its PSUM output starts at a nonzero column
offset; all offset-256 matmuls are therefore split into <=64-weight
sub-matmuls.
"""

import sys

# concourse normally comes from the axon site tree (sitecustomize); the
# append is a fallback so a bare environment still finds it.
if "/opt/trn_rl_repo" not in sys.path:
    sys.path.append("/opt/trn_rl_repo")

import numpy as np

import concourse.bass as bass  # noqa: F401  (engine namespaces live on nc)
from concourse.bass import _add_dep_helper
import concourse.mybir as mybir
import concourse.tile as tile
from concourse import bacc
from concourse.bass_utils import run_bass_kernel_spmd

F32 = mybir.dt.float32
BF16 = mybir.dt.bfloat16
AF = mybir.ActivationFunctionType

# problem dims (hardcoded per contract)
B = 8
F = 16
P = 196
D = 768
NH = 12
HD = 64
N = F * P  # 3136
E3 = 3 * D  # 2304
SB = 784  # superblock = lcm(196, 16) tokens
NSB = N // SB  # 4
FPSB = SB // P  # 4 frames per superblock
WPSB = SB // 112  # 7 temporal windows per superblock
SCALE = HD ** -0.5

# compute dtype for matmul inputs ("f32" safest, "bf16" 4x faster on PE)
COMPUTE = "bf16"

_CACHE = {}


def _build(compute: str, reps: int = 1, ncores: int = B):
    """Build + bass-compile the per-core kernel. Returns the Bacc object.

    compute: "f32" | "bf16" — dtype of all matmul inputs.
    reps: device-side repetition count (for timing; wraps the body in For_i).
    """
    cdt = BF16 if compute == "bf16" else F32

    nc = bacc.Bacc("TRN2", target_bir_lowering=False, debug=False,
                   num_devices=ncores)

    xt_d = nc.dram_tensor("xt", (D, N), cdt, kind="ExternalInput")
    wqkv_d = nc.dram_tensor("wqkvT", (D, E3), cdt, kind="ExternalInput")
    wproj_d = nc.dram_tensor("wprojT", (D, D), cdt, kind="ExternalInput")
    bias_d = nc.dram_tensor("bias", (D, 1), F32, kind="ExternalInput")
    mask_d = nc.dram_tensor("mask", (112, 112), cdt, kind="ExternalInput")
    out_d = nc.dram_tensor("outT", (D, N), F32, kind="ExternalOutput")

    with tile.TileContext(nc) as tc:
        with (
            tc.tile_pool(name="const", bufs=1) as cpool,
            tc.tile_pool(name="work", bufs=1) as wpool,
            tc.tile_pool(name="small", bufs=4) as spool,
            tc.tile_pool(name="psum", bufs=2, space="PSUM") as ppool,
        ):
            # ---- constants -------------------------------------------------
            wq = []
            for dc in range(6):
                t = cpool.tile([128, E3], cdt, tag=f"wq{dc}", name=f"wq{dc}")
                nc.sync.dma_start(t[:], wqkv_d.ap()[128 * dc : 128 * (dc + 1), :])
                wq.append(t)
            wp = []
            for dc in range(6):
                t = cpool.tile([128, D], cdt, tag=f"wp{dc}", name=f"wp{dc}")
                nc.sync.dma_start(t[:], wproj_d.ap()[128 * dc : 128 * (dc + 1), :])
                wp.append(t)
            bias_t = cpool.tile([128, 6], F32, tag="bias", name="bias_t")
            nc.sync.dma_start(
                bias_t[:], bias_d.ap().rearrange("(e p) one -> p (e one)", p=128)
            )
            # block-diag mask replicated 2x along free dim: one masked-mul
            # per head-pair temporal scores tile
            mask4_t = cpool.tile([112, 224], cdt, tag="mask", name="mask4_t")
            for r in range(2):
                nc.sync.dma_start(mask4_t[:, 112 * r : 112 * (r + 1)], mask_d.ap())
            zeros_col = cpool.tile([128, 1], F32, tag="zeros_c", name="zeros_col")
            nc.gpsimd.memset(zeros_col[:], 0.0)
            # all-ones stationary operand of the denominator matmuls
            ones_t = cpool.tile([128, 64], cdt, tag="ones", name="ones_t")
            nc.gpsimd.memset(ones_t[:], 1.0)

            import contextlib

            rep_ctx = tc.For_i(0, reps, 1) if reps > 1 else contextlib.nullcontext()
            with rep_ctx:
              for s in range(NSB):
                so = SB * s  # superblock token offset

                # ---- load x^T superblock ----------------------------------
                xts = []
                for dc in range(6):
                    t = wpool.tile([128, SB], cdt, tag=f"xts{dc}", bufs=2,
                                   name=f"xts{dc}_{s}")
                    nc.sync.dma_start(
                        t[:], xt_d.ap()[128 * dc : 128 * (dc + 1), so : so + SB]
                    )
                    xts.append(t)

                # ---- QKV projection: Q,K regions, feature-major -----------
                # qkvt[t] rows = features 128t..128t+127 of [Q(768) | K(768)]
                qkvt = []
                for ti in range(12):
                    qt = wpool.tile([128, SB], cdt, tag=f"qkvt{ti}", bufs=2,
                                    name=f"qkvt{ti}_{s}")
                    for j in range(2):
                        ps = ppool.tile([128, 392], F32, tag="mm", bufs=2,
                                        name=f"ps_qk{s}_{ti}_{j}")
                        for dc in range(6):
                            nc.tensor.matmul(
                                ps[:],
                                wq[dc][:, 128 * ti : 128 * (ti + 1)],
                                xts[dc][:, 392 * j : 392 * (j + 1)],
                                start=(dc == 0),
                                stop=(dc == 5),
                            )
                        # split evacuation between the two engines
                        if j == 0:
                            nc.scalar.copy(qt[:, 0:392], ps[:])
                        else:
                            nc.vector.tensor_copy(qt[:, 392:784], ps[:])
                    qkvt.append(qt)

                # ---- V projection, token-major (natural) ------------------
                # per tile: 6 heads x 64 V-cols = 384 cols
                def v_proj(msz, tok0, wcol0, vtag, vname, psname):
                    vt_ = wpool.tile([msz, 384], cdt, tag=vtag, bufs=2, name=vname)
                    ps = ppool.tile([msz, 384], F32, tag="mm", bufs=2, name=psname)
                    for dc in range(6):
                        nc.tensor.matmul(
                            ps[:],
                            xts[dc][:, tok0 : tok0 + msz],
                            wq[dc][:, wcol0 : wcol0 + 384],
                            start=(dc == 0),
                            stop=(dc == 5),
                        )
                    nc.scalar.copy(vt_[:], ps[:])
                    return vt_

                # spatial V: per-frame chunks of [128+68] rows; cols = heads 0-5
                vs = []
                for f in range(FPSB):
                    for ci, (m0, msz) in enumerate(((0, 128), (128, 68))):
                        vs.append(
                            v_proj(msz, 196 * f + m0, 1536, f"vs{f}_{ci}",
                                   f"vs{f}_{ci}_{s}", f"ps_vs{s}_{f}_{ci}")
                        )
                # temporal V: uniform 112-token windows; cols = heads 6-11
                vt = []
                for w in range(WPSB):
                    vt.append(
                        v_proj(112, 112 * w, 1920, f"vt{w}",
                               f"vt{w}_{s}", f"ps_vt{s}_{w}")
                    )

                # ---- attention output, feature-major ----------------------
                attnT = [
                    wpool.tile([128, SB], cdt, tag=f"attnT{i}", bufs=2,
                               name=f"attnT{i}_{s}")
                    for i in range(6)
                ]

                def pair_norm(ps_avd, at, cols, L, name):
                    """reciprocal of den + one multiply -> attnT columns.

                    Data deps keep this bank-collision safe: the matmul
                    chain ends with the den groups, reciprocal RAW-depends
                    on them, and the multiply RAW-depends on rb — so no
                    engine reads the bank while the PE still writes it.
                    """
                    rb = spool.tile([128, L], F32, tag="rb", name=f"rb{name}")
                    nc.vector.reciprocal(rb[:], ps_avd[:, 256 : 256 + L])
                    nc.vector.tensor_mul(at[:, cols], ps_avd[:, 0:L], rb[:])

                # chain same-bank psum groups sequentially: a group's
                # start=True clears the bank's has_written bits, so it must
                # not begin until the previous group in that bank stopped.
                def chain(mm, prev):
                    if prev is not None:
                        _add_dep_helper(mm.ins, prev.ins, sync=False,
                                        reason="sequential psum groups in bank")
                    return mm

                # ---- spatial attention (heads 0-5, by pair, per frame) -----
                import os

                _abl = os.environ.get("KABL", "st")
                _slvl = 3 if "s" in _abl else 0
                for p in _abl.split(","):
                    if p.startswith("s") and p[1:].isdigit():
                        _slvl = int(p[1:])
                if _abl != "st":
                    for t in attnT:
                        nc.gpsimd.memset(t[:], 0.0)
                for f in range(FPSB) if _slvl >= 1 else []:
                    fo = 196 * f
                    for hp in range(3):
                        qtile = qkvt[hp]       # Q features, heads 2hp/2hp+1
                        ktile = qkvt[6 + hp]   # K features
                        es = {}
                        for ci, (m0, msz) in enumerate(((0, 128), (128, 68))):
                            ps_st = ppool.tile(
                                [msz, 512], F32, tag="st", bufs=3,
                                name=f"ps_st{s}_{f}_{hp}_{ci}",
                            )
                            # HW erratum: a K<128 matmul with >64 weight
                            # columns crashes when its PSUM output starts at
                            # a column offset, so matmuls at offset 256 are
                            # split into <=64-weight sub-matmuls.
                            for hi in range(2):
                                pb = 64 * hi
                                co = 256 * hi
                                pieces = (
                                    ((0, msz),) if hi == 0
                                    else ((0, 64), (64, msz - 64))
                                )
                                for c0, csz in pieces:
                                    nc.tensor.matmul(
                                        ps_st[c0 : c0 + csz, co : co + 196],
                                        ktile[pb : pb + 64,
                                              fo + m0 + c0 : fo + m0 + c0 + csz],
                                        qtile[pb : pb + 64, fo : fo + 196],
                                        start=True,
                                        stop=True,
                                        skip_group_check=(c0 > 0),
                                    )
                            e = spool.tile(
                                [msz, 392], cdt, tag="e", bufs=8,
                                name=f"e{s}_{f}_{hp}_{ci}",
                            )
                            nc.scalar.activation(
                                e.rearrange("p (g c) -> p g c", c=196),
                                ps_st.rearrange("p (g c) -> p g c", c=256)[
                                    :, :, 0:196
                                ],
                                AF.Exp,
                                bias=zeros_col[:msz, :], scale=SCALE,
                            )
                            es[ci] = e
                        if _slvl < 2:
                            continue
                        ps_avd = ppool.tile([128, 512], F32, tag="avd", bufs=3,
                                            name=f"ps_sav{s}_{f}_{hp}")
                        prev = None  # last matmul of the previous group
                        for kind in range(2):  # 0=num @0, 1=den @256
                            for hi in range(2):
                                h = 2 * hp + hi
                                rs = slice(64 * hi, 64 * hi + 64)
                                co = 256 * kind
                                for ci, (m0, msz) in enumerate(((0, 128), (128, 68))):
                                    lhsT = (
                                        vs[2 * f + ci][:, 64 * h : 64 * h + 64]
                                        if kind == 0
                                        else ones_t[:msz, :]
                                    )
                                    mm = nc.tensor.matmul(
                                        ps_avd[rs, co : co + 196],
                                        lhsT,
                                        es[ci][:msz, 196 * hi : 196 * hi + 196],
                                        start=(ci == 0),
                                        stop=(ci == 1),
                                        skip_group_check=(hi == 1),
                                    )
                                    if ci == 0:
                                        chain(mm, prev)
                                    if ci == 1:
                                        prev = mm
                        if _slvl < 3:
                            continue
                        pair_norm(ps_avd, attnT[hp],
                                  slice(fo, fo + 196), 196, f"s{s}_{f}_{hp}")

                # ---- temporal attention (heads 6-11, per 112-window) -------
                # block-diag mask folded into the score matmul (rank-8 term)
                for w in range(WPSB) if "t" in _abl else []:
                    wo = 112 * w
                    for hp in range(3):
                        ps_st = ppool.tile(
                            [112, 512], F32, tag="st", bufs=3,
                            name=f"ps_tst{s}_{w}_{hp}",
                        )
                        for hi in range(2):
                            pb = 64 * hi
                            co = 256 * hi
                            # offset-256 matmuls split to <=64 weight cols
                            # (see spatial erratum comment)
                            pieces = (
                                ((0, 112),) if hi == 0 else ((0, 64), (64, 48))
                            )
                            for c0, csz in pieces:
                                nc.tensor.matmul(
                                    ps_st[c0 : c0 + csz, co : co + 112],
                                    qkvt[9 + hp][pb : pb + 64,
                                                 wo + c0 : wo + c0 + csz],
                                    qkvt[3 + hp][pb : pb + 64, wo : wo + 112],
                                    start=True,
                                    stop=True,
                                    skip_group_check=(c0 > 0),
                                )
                        e = spool.tile(
                            [112, 224], cdt, tag="e", bufs=8,
                            name=f"et{s}_{w}_{hp}",
                        )
                        nc.scalar.activation(
                            e.rearrange("p (g c) -> p g c", c=112),
                            ps_st.rearrange("p (g c) -> p g c", c=256)[:, :, 0:112],
                            AF.Exp,
                            bias=zeros_col[:112, :], scale=SCALE,
                        )
                        em = spool.tile(
                            [112, 224], cdt, tag="em", bufs=6,
                            name=f"emt{s}_{w}_{hp}",
                        )
                        nc.vector.tensor_mul(em[:], e[:], mask4_t[:])
                        ps_avd = ppool.tile([128, 512], F32, tag="avd", bufs=3,
                                            name=f"ps_tav{s}_{w}_{hp}")
                        prev = None
                        for kind in range(2):  # 0=num @0, 1=den @256
                            for hi in range(2):
                                h = 6 + 2 * hp + hi
                                rs = slice(64 * hi, 64 * hi + 64)
                                co = 256 * kind
                                lhsT = (
                                    vt[w][:, 64 * (h - 6) : 64 * (h - 6) + 64]
                                    if kind == 0
                                    else ones_t[:112, :]
                                )
                                mm = nc.tensor.matmul(
                                    ps_avd[rs, co : co + 112],
                                    lhsT,
                                    em[:112, 112 * hi : 112 * hi + 112],
                                    start=True,
                                    stop=True,
                                    skip_group_check=(hi == 1),
                                )
                                chain(mm, prev)
                                prev = mm
                        pair_norm(ps_avd, attnT[3 + hp],
                                  slice(wo, wo + 112), 112, f"t{s}_{w}_{hp}")

                # ---- output projection + bias ------------------------------
                for ec in range(6):
                    for j in range(2):
                        ps = ppool.tile([128, 392], F32, tag="mm", bufs=2,
                                        name=f"ps_o{s}_{ec}_{j}")
                        for dc in range(6):
                            nc.tensor.matmul(
                                ps[:],
                                wp[dc][:, 128 * ec : 128 * (ec + 1)],
                                attnT[dc][:, 392 * j : 392 * (j + 1)],
                                start=(dc == 0),
                                stop=(dc == 5),
                            )
                        ot = spool.tile([128, 392], F32, tag="ot",
                                        name=f"ot{s}_{ec}_{j}")
                        nc.scalar.activation(
                            ot[:], ps[:], AF.Identity,
                            bias=bias_t[:, ec : ec + 1], scale=1.0,
                        )
                        nc.sync.dma_start(
                            out_d.ap()[
                                128 * ec : 128 * (ec + 1),
                                so + 392 * j : so + 392 * (j + 1),
                            ],
                            ot[:],
                        )

    nc.compile()
    return nc


def _get_nc(compute: str):
    if compute not in _CACHE:
        _CACHE[compute] = _build(compute)
    return _CACHE[compute]


def _np_dtype(compute: str):
    if compute == "f32":
        return np.float32
    import ml_dtypes

    return ml_dtypes.bfloat16


def _mask_np(dt):
    """Block-diagonal 0/1 mask for the temporal 16-token groups."""
    mask = np.zeros((112, 112), np.float32)
    for g in range(7):
        mask[16 * g : 16 * (g + 1), 16 * g : 16 * (g + 1)] = 1.0
    return mask.astype(dt)


def _host_inputs(x, w_qkv, w_proj, b_proj, dt):
    """Full inputs -> per-core in_maps (data-parallel over batch)."""
    x = np.asarray(x, dtype=np.float32).reshape(B, N, D)
    xT = np.ascontiguousarray(x.transpose(0, 2, 1)).astype(dt)  # (B, D, N)
    wqkvT = np.ascontiguousarray(np.asarray(w_qkv, np.float32).T).astype(dt)
    wprojT = np.ascontiguousarray(np.asarray(w_proj, np.float32).T).astype(dt)
    bias = np.asarray(b_proj, np.float32).reshape(D, 1)
    return [
        {"xt": xT[b], "wqkvT": wqkvT, "wprojT": wprojT, "bias": bias,
         "mask": _mask_np(dt)}
        for b in range(B)
    ]


def kernel(x, w_qkv, w_proj, b_proj):
    nc = _get_nc(COMPUTE)
    dt = _np_dtype(COMPUTE)
    in_maps = _host_inputs(x, w_qkv, w_proj, b_proj, dt)
    res = run_bass_kernel_spmd(nc, in_maps, core_ids=list(range(B)))
    out = np.stack([r["outT"].T for r in res.results])  # (B, N, D)
    return np.ascontiguousarray(out.reshape(B, F, P, D)).astype(np.float32)


if __name__ == "__main__":
    rng = np.random.default_rng(0)
    x = rng.standard_normal((B, F, P, D), dtype=np.float32)
    w_qkv = rng.standard_normal((E3, D), dtype=np.float32) * D**-0.5
    w_proj = rng.standard_normal((D, D), dtype=np.float32) * D**-0.5
    b_proj = np.zeros(D, np.float32)
    out = kernel(x=x, w_qkv=w_qkv, w_proj=w_proj, b_proj=b_proj)
    print(out.shape, out.dtype)


# revision 54
# speedup vs baseline: 1.5093x; 1.2051x over previous
"""Trainium2 Bass kernel for factorized space-time attention.

Computation (per batch b of 8, one NeuronCore each):
  qkv = x @ w_qkv.T                      (3136, 2304)
  heads 0-5:  spatial attention over 196 patches within each of 16 frames
  heads 6-11: temporal attention over groups of 16 consecutive tokens
              (raw-reshape semantics of the reference)
  out = concat(head outputs) @ w_proj.T + b_proj

Strategy: data-parallel over batch (8 cores). All activations kept
feature-major ([d, n]) on chip so every matmul contraction runs over the
partition dim with no on-device transposes; x / weights are pre-transposed
host-side and cast to bf16 (PE runs bf16 at 4x the fp32 rate; tolerance is
2e-2 so bf16 rounding is comfortably inside budget).

Attention works on head PAIRS so normalization is partition-aligned.
Matmul PSUM outputs must start at a 1024-byte (256 fp32 column) boundary
inside their bank (an unaligned column offset crashes the device), so
pair tiles pack two regions per bank at column offsets 0 and 256:
  ps_st  [msz, 512]  scores: even head @0, odd head @256 (one exp inst
                     reads both via a strided AP)
  ps_avd [128, 512]  AV numerators @0 (rows 0:64 even / 64:128 odd) and
                     softmax denominators @256, broadcast to the same rows
                     via an extra ones-matmul
  one DVE reciprocal [128, L] + one DVE multiply write the normalized pair
  straight into attnT — no broadcast matmul, no scalar-engine staging copy,
  no partition-shift DMA.
Temporal attention exp()s the raw 112x112 window scores, then one DVE
multiply with the replicated block-diagonal 0/1 mask zeroes the
off-diagonal groups before the AV / denominator matmuls.
Softmax skips the max-subtraction (scores are ~N(0,1); exp is safe).
HW erratum found while packing: a K<128 matmul with >64 weight columns
crashes the device when its PSUM output starts at a nonzero column
offset; all offset-256 matmuls are therefore split into <=64-weight
sub-matmuls.
"""

import sys

# concourse normally comes from the axon site tree (sitecustomize); the
# append is a fallback so a bare environment still finds it.
if "/opt/trn_rl_repo" not in sys.path:
    sys.path.append("/opt/trn_rl_repo")

import numpy as np

import concourse.bass as bass  # noqa: F401  (engine namespaces live on nc)
from concourse.bass import _add_dep_helper
import concourse.mybir as mybir
import concourse.tile as tile
from concourse import bacc
from concourse.bass_utils import run_bass_kernel_spmd

F32 = mybir.dt.float32
BF16 = mybir.dt.bfloat16
AF = mybir.ActivationFunctionType

# problem dims (hardcoded per contract)
B = 8
F = 16
P = 196
D = 768
NH = 12
HD = 64
N = F * P  # 3136
E3 = 3 * D  # 2304
SB = 784  # superblock = lcm(196, 16) tokens
NSB = N // SB  # 4
FPSB = SB // P  # 4 frames per superblock
WPSB = SB // 112  # 7 temporal windows per superblock
SCALE = HD ** -0.5

# compute dtype for matmul inputs ("f32" safest, "bf16" 4x faster on PE)
COMPUTE = "bf16"

_CACHE = {}


def _build(compute: str, reps: int = 1, ncores: int = B):
    """Build + bass-compile the per-core kernel. Returns the Bacc object.

    compute: "f32" | "bf16" — dtype of all matmul inputs.
    reps: device-side repetition count (for timing; wraps the body in For_i).
    """
    cdt = BF16 if compute == "bf16" else F32

    nc = bacc.Bacc("TRN2", target_bir_lowering=False, debug=False,
                   num_devices=ncores)

    xt_d = nc.dram_tensor("xt", (D, N), cdt, kind="ExternalInput")
    wqkv_d = nc.dram_tensor("wqkvT", (D, E3), cdt, kind="ExternalInput")
    wproj_d = nc.dram_tensor("wprojT", (D, D), cdt, kind="ExternalInput")
    bias_d = nc.dram_tensor("bias", (D, 1), F32, kind="ExternalInput")
    mask_d = nc.dram_tensor("mask", (112, 112), cdt, kind="ExternalInput")
    out_d = nc.dram_tensor("outT", (D, N), F32, kind="ExternalOutput")

    with tile.TileContext(nc) as tc:
        with (
            tc.tile_pool(name="const", bufs=1) as cpool,
            tc.tile_pool(name="work", bufs=1) as wpool,
            tc.tile_pool(name="small", bufs=4) as spool,
            tc.tile_pool(name="psum", bufs=2, space="PSUM") as ppool,
        ):
            # ---- constants -------------------------------------------------
            wq = []
            for dc in range(6):
                t = cpool.tile([128, E3], cdt, tag=f"wq{dc}", name=f"wq{dc}")
                nc.sync.dma_start(t[:], wqkv_d.ap()[128 * dc : 128 * (dc + 1), :])
                wq.append(t)
            wp = []
            for dc in range(6):
                t = cpool.tile([128, D], cdt, tag=f"wp{dc}", name=f"wp{dc}")
                nc.sync.dma_start(t[:], wproj_d.ap()[128 * dc : 128 * (dc + 1), :])
                wp.append(t)
            bias_t = cpool.tile([128, 6], F32, tag="bias", name="bias_t")
            nc.sync.dma_start(
                bias_t[:], bias_d.ap().rearrange("(e p) one -> p (e one)", p=128)
            )
            # block-diag mask replicated 2x along free dim: one masked-mul
            # per head-pair temporal scores tile
            mask4_t = cpool.tile([112, 224], cdt, tag="mask", name="mask4_t")
            for r in range(2):
                nc.sync.dma_start(mask4_t[:, 112 * r : 112 * (r + 1)], mask_d.ap())
            zeros_col = cpool.tile([128, 1], F32, tag="zeros_c", name="zeros_col")
            nc.gpsimd.memset(zeros_col[:], 0.0)
            # all-ones stationary operand of the denominator matmuls
            ones_t = cpool.tile([128, 64], cdt, tag="ones", name="ones_t")
            nc.gpsimd.memset(ones_t[:], 1.0)

            import contextlib

            rep_ctx = tc.For_i(0, reps, 1) if reps > 1 else contextlib.nullcontext()
            with rep_ctx:
              for s in range(NSB):
                so = SB * s  # superblock token offset

                # ---- load x^T superblock ----------------------------------
                xts = []
                for dc in range(6):
                    t = wpool.tile([128, SB], cdt, tag=f"xts{dc}", bufs=2,
                                   name=f"xts{dc}_{s}")
                    nc.sync.dma_start(
                        t[:], xt_d.ap()[128 * dc : 128 * (dc + 1), so : so + SB]
                    )
                    xts.append(t)

                # ---- QKV projection: Q,K regions, feature-major -----------
                # qkvt[t] rows = features 128t..128t+127 of [Q(768) | K(768)]
                qkvt = []
                for ti in range(12):
                    qt = wpool.tile([128, SB], cdt, tag=f"qkvt{ti}", bufs=3,
                                    name=f"qkvt{ti}_{s}")
                    for j in range(2):
                        ps = ppool.tile([128, 392], F32, tag="mm", bufs=2,
                                        name=f"ps_qk{s}_{ti}_{j}")
                        for dc in range(6):
                            nc.tensor.matmul(
                                ps[:],
                                wq[dc][:, 128 * ti : 128 * (ti + 1)],
                                xts[dc][:, 392 * j : 392 * (j + 1)],
                                start=(dc == 0),
                                stop=(dc == 5),
                            )
                        # split evacuation between the two engines
                        if j == 0:
                            nc.scalar.copy(qt[:, 0:392], ps[:])
                        else:
                            nc.vector.tensor_copy(qt[:, 392:784], ps[:])
                    qkvt.append(qt)

                # ---- V projection, token-major (natural) ------------------
                # per tile: 6 heads x 64 V-cols = 384 cols
                def v_proj(msz, tok0, wcol0, vtag, vname, psname):
                    vt_ = wpool.tile([msz, 384], cdt, tag=vtag, bufs=2, name=vname)
                    ps = ppool.tile([msz, 384], F32, tag="mm", bufs=2, name=psname)
                    for dc in range(6):
                        nc.tensor.matmul(
                            ps[:],
                            xts[dc][:, tok0 : tok0 + msz],
                            wq[dc][:, wcol0 : wcol0 + 384],
                            start=(dc == 0),
                            stop=(dc == 5),
                        )
                    nc.scalar.copy(vt_[:], ps[:])
                    return vt_

                # spatial V: per-frame chunks of [128+68] rows; cols = heads 0-5
                vs = []
                for f in range(FPSB):
                    for ci, (m0, msz) in enumerate(((0, 128), (128, 68))):
                        vs.append(
                            v_proj(msz, 196 * f + m0, 1536, f"vs{f}_{ci}",
                                   f"vs{f}_{ci}_{s}", f"ps_vs{s}_{f}_{ci}")
                        )
                # temporal V: uniform 112-token windows; cols = heads 6-11
                vt = []
                for w in range(WPSB):
                    vt.append(
                        v_proj(112, 112 * w, 1920, f"vt{w}",
                               f"vt{w}_{s}", f"ps_vt{s}_{w}")
                    )

                # ---- attention output, feature-major ----------------------
                attnT = [
                    wpool.tile([128, SB], cdt, tag=f"attnT{i}", bufs=3,
                               name=f"attnT{i}_{s}")
                    for i in range(6)
                ]

                def pair_norm(ps_avd, at, cols, L, name):
                    """reciprocal of den + one multiply -> attnT columns.

                    Data deps keep this bank-collision safe: the matmul
                    chain ends with the den groups, reciprocal RAW-depends
                    on them, and the multiply RAW-depends on rb — so no
                    engine reads the bank while the PE still writes it.
                    """
                    rb = spool.tile([128, L], F32, tag="rb", name=f"rb{name}")
                    nc.vector.reciprocal(rb[:], ps_avd[:, 256 : 256 + L])
                    nc.vector.tensor_mul(at[:, cols], ps_avd[:, 0:L], rb[:])

                # chain same-bank psum groups sequentially: a group's
                # start=True clears the bank's has_written bits, so it must
                # not begin until the previous group in that bank stopped.
                def chain(mm, prev):
                    if prev is not None:
                        _add_dep_helper(mm.ins, prev.ins, sync=False,
                                        reason="sequential psum groups in bank")
                    return mm

                # ---- spatial attention (heads 0-5, by pair, per frame) -----
                for f in range(FPSB):
                    fo = 196 * f
                    for hp in range(3):
                        qtile = qkvt[hp]       # Q features, heads 2hp/2hp+1
                        ktile = qkvt[6 + hp]   # K features
                        es = {}
                        for ci, (m0, msz) in enumerate(((0, 128), (128, 68))):
                            ps_st = ppool.tile(
                                [msz, 512], F32, tag="st", bufs=3,
                                name=f"ps_st{s}_{f}_{hp}_{ci}",
                            )
                            # HW erratum: a K<128 matmul with >64 weight
                            # columns crashes when its PSUM output starts at
                            # a column offset, so matmuls at offset 256 are
                            # split into <=64-weight sub-matmuls.
                            for hi in range(2):
                                pb = 64 * hi
                                co = 256 * hi
                                pieces = (
                                    ((0, msz),) if hi == 0
                                    else ((0, 64), (64, msz - 64))
                                )
                                for c0, csz in pieces:
                                    nc.tensor.matmul(
                                        ps_st[c0 : c0 + csz, co : co + 196],
                                        ktile[pb : pb + 64,
                                              fo + m0 + c0 : fo + m0 + c0 + csz],
                                        qtile[pb : pb + 64, fo : fo + 196],
                                        start=True,
                                        stop=True,
                                        skip_group_check=(c0 > 0),
                                    )
                            e = spool.tile(
                                [msz, 392], cdt, tag="e", bufs=8,
                                name=f"e{s}_{f}_{hp}_{ci}",
                            )
                            nc.scalar.activation(
                                e.rearrange("p (g c) -> p g c", c=196),
                                ps_st.rearrange("p (g c) -> p g c", c=256)[
                                    :, :, 0:196
                                ],
                                AF.Exp,
                                bias=zeros_col[:msz, :], scale=SCALE,
                            )
                            es[ci] = e
                        ps_avd = ppool.tile([128, 512], F32, tag="avd", bufs=3,
                                            name=f"ps_sav{s}_{f}_{hp}")
                        prev = None  # last matmul of the previous group
                        for kind in range(2):  # 0=num @0, 1=den @256
                            for hi in range(2):
                                h = 2 * hp + hi
                                rs = slice(64 * hi, 64 * hi + 64)
                                co = 256 * kind
                                for ci, (m0, msz) in enumerate(((0, 128), (128, 68))):
                                    lhsT = (
                                        vs[2 * f + ci][:, 64 * h : 64 * h + 64]
                                        if kind == 0
                                        else ones_t[:msz, :]
                                    )
                                    mm = nc.tensor.matmul(
                                        ps_avd[rs, co : co + 196],
                                        lhsT,
                                        es[ci][:msz, 196 * hi : 196 * hi + 196],
                                        start=(ci == 0),
                                        stop=(ci == 1),
                                        skip_group_check=(hi == 1),
                                    )
                                    if ci == 0:
                                        chain(mm, prev)
                                    if ci == 1:
                                        prev = mm
                        pair_norm(ps_avd, attnT[hp],
                                  slice(fo, fo + 196), 196, f"s{s}_{f}_{hp}")

                # ---- temporal attention (heads 6-11, per 112-window) -------
                # block-diag mask folded into the score matmul (rank-8 term)
                for w in range(WPSB):
                    wo = 112 * w
                    for hp in range(3):
                        ps_st = ppool.tile(
                            [112, 512], F32, tag="st", bufs=3,
                            name=f"ps_tst{s}_{w}_{hp}",
                        )
                        for hi in range(2):
                            pb = 64 * hi
                            co = 256 * hi
                            # offset-256 matmuls split to <=64 weight cols
                            # (see spatial erratum comment)
                            pieces = (
                                ((0, 112),) if hi == 0 else ((0, 64), (64, 48))
                            )
                            for c0, csz in pieces:
                                nc.tensor.matmul(
                                    ps_st[c0 : c0 + csz, co : co + 112],
                                    qkvt[9 + hp][pb : pb + 64,
                                                 wo + c0 : wo + c0 + csz],
                                    qkvt[3 + hp][pb : pb + 64, wo : wo + 112],
                                    start=True,
                                    stop=True,
                                    skip_group_check=(c0 > 0),
                                )
                        e = spool.tile(
                            [112, 224], cdt, tag="e", bufs=8,
                            name=f"et{s}_{w}_{hp}",
                        )
                        nc.scalar.activation(
                            e.rearrange("p (g c) -> p g c", c=112),
                            ps_st.rearrange("p (g c) -> p g c", c=256)[:, :, 0:112],
                            AF.Exp,
                            bias=zeros_col[:112, :], scale=SCALE,
                        )
                        em = spool.tile(
                            [112, 224], cdt, tag="em", bufs=6,
                            name=f"emt{s}_{w}_{hp}",
                        )
                        nc.vector.tensor_mul(em[:], e[:], mask4_t[:])
                        ps_avd = ppool.tile([128, 512], F32, tag="avd", bufs=3,
                                            name=f"ps_tav{s}_{w}_{hp}")
                        prev = None
                        for kind in range(2):  # 0=num @0, 1=den @256
                            for hi in range(2):
                                h = 6 + 2 * hp + hi
                                rs = slice(64 * hi, 64 * hi + 64)
                                co = 256 * kind
                                lhsT = (
                                    vt[w][:, 64 * (h - 6) : 64 * (h - 6) + 64]
                                    if kind == 0
                                    else ones_t[:112, :]
                                )
                                mm = nc.tensor.matmul(
                                    ps_avd[rs, co : co + 112],
                                    lhsT,
                                    em[:112, 112 * hi : 112 * hi + 112],
                                    start=True,
                                    stop=True,
                                    skip_group_check=(hi == 1),
                                )
                                chain(mm, prev)
                                prev = mm
                        pair_norm(ps_avd, attnT[3 + hp],
                                  slice(wo, wo + 112), 112, f"t{s}_{w}_{hp}")

                # ---- output projection + bias ------------------------------
                for ec in range(6):
                    for j in range(2):
                        ps = ppool.tile([128, 392], F32, tag="mm", bufs=2,
                                        name=f"ps_o{s}_{ec}_{j}")
                        for dc in range(6):
                            nc.tensor.matmul(
                                ps[:],
                                wp[dc][:, 128 * ec : 128 * (ec + 1)],
                                attnT[dc][:, 392 * j : 392 * (j + 1)],
                                start=(dc == 0),
                                stop=(dc == 5),
                            )
                        ot = spool.tile([128, 392], F32, tag="ot",
                                        name=f"ot{s}_{ec}_{j}")
                        nc.scalar.activation(
                            ot[:], ps[:], AF.Identity,
                            bias=bias_t[:, ec : ec + 1], scale=1.0,
                        )
                        nc.sync.dma_start(
                            out_d.ap()[
                                128 * ec : 128 * (ec + 1),
                                so + 392 * j : so + 392 * (j + 1),
                            ],
                            ot[:],
                        )

    nc.compile()
    return nc


def _get_nc(compute: str):
    if compute not in _CACHE:
        _CACHE[compute] = _build(compute)
    return _CACHE[compute]


def _np_dtype(compute: str):
    if compute == "f32":
        return np.float32
    import ml_dtypes

    return ml_dtypes.bfloat16


def _mask_np(dt):
    """Block-diagonal 0/1 mask for the temporal 16-token groups."""
    mask = np.zeros((112, 112), np.float32)
    for g in range(7):
        mask[16 * g : 16 * (g + 1), 16 * g : 16 * (g + 1)] = 1.0
    return mask.astype(dt)


def _host_inputs(x, w_qkv, w_proj, b_proj, dt):
    """Full inputs -> per-core in_maps (data-parallel over batch)."""
    x = np.asarray(x, dtype=np.float32).reshape(B, N, D)
    xT = np.ascontiguousarray(x.transpose(0, 2, 1)).astype(dt)  # (B, D, N)
    wqkvT = np.ascontiguousarray(np.asarray(w_qkv, np.float32).T).astype(dt)
    wprojT = np.ascontiguousarray(np.asarray(w_proj, np.float32).T).astype(dt)
    bias = np.asarray(b_proj, np.float32).reshape(D, 1)
    return [
        {"xt": xT[b], "wqkvT": wqkvT, "wprojT": wprojT, "bias": bias,
         "mask": _mask_np(dt)}
        for b in range(B)
    ]


def kernel(x, w_qkv, w_proj, b_proj):
    nc = _get_nc(COMPUTE)
    dt = _np_dtype(COMPUTE)
    in_maps = _host_inputs(x, w_qkv, w_proj, b_proj, dt)
    res = run_bass_kernel_spmd(nc, in_maps, core_ids=list(range(B)))
    out = np.stack([r["outT"].T for r in res.results])  # (B, N, D)
    return np.ascontiguousarray(out.reshape(B, F, P, D)).astype(np.float32)


if __name__ == "__main__":
    rng = np.random.default_rng(0)
    x = rng.standard_normal((B, F, P, D), dtype=np.float32)
    w_qkv = rng.standard_normal((E3, D), dtype=np.float32) * D**-0.5
    w_proj = rng.standard_normal((D, D), dtype=np.float32) * D**-0.5
    b_proj = np.zeros(D, np.float32)
    out = kernel(x=x, w_qkv=w_qkv, w_proj=w_proj, b_proj=b_proj)
    print(out.shape, out.dtype)
